# revision 4
# baseline (speedup 1.0000x reference)
"""Causal multi-head self-attention with RoPE — Trainium2 Bass kernel.

Problem: B=2, S=2048, D=1024, H=16 heads, dk=64, fp32.
Sharding: 32 (batch, head) units -> 8 cores x (2 heads x 2 batches).
  - Wq/Wk/Wv split column-wise (by head) per core; Wo split row-wise; the
    8 partial outputs are summed on the host (the row-split Wo partial sums).
  - Host pre-transposes x -> x^T and reorders Wq/Wk rows within each head to
    even-first/odd-second ("half-split") so that RoPE on device becomes a
    32-partition block-swap + elementwise ops (scores are invariant to a
    shared permutation of q and k head dims).

Per-core device pipeline (per batch):
  xT chunks -> Q^T/K^T/V^T projections (dk on partitions, tokens free)
  RoPE: qrot = q*cos + blockswap32(q*spre)   (spre = sign-corrected sin)
  V: PE-transpose to token-partition layout, with a ones column appended
  scores^T[k,q] = matmul(lhsT=K[dk,kt], rhs=Q[dk,qt]) (+ causal mask added
    via identity-matmul accumulation into PSUM), exp on ScalarE
  out^T[dv,q] = matmul(lhsT=V_aug[kt, 65], rhs=exp^T[kt, qt]) accumulated
    over kt; row 64 = softmax denominators (ones column trick)
  normalize via reciprocal + gpsimd partition-broadcast, stack heads,
  y = U_norm^T.T @ Wo_slice with per-token scaling already applied.
"""

import sys
import os

sys.path.insert(0, "/opt/trn_rl_repo")

import numpy as np

import concourse.bass as bass
import concourse.tile as tile
import concourse.mybir as mybir
from concourse import bacc
from concourse.masks import make_identity

# ---------------------------------------------------------------- constants
B = 2
S = 2048
D = 1024
H = 16
DK = 64
THETA = 10000.0
NCORES = 8
HLOC = H // NCORES          # heads per core = 2
P = 128
CH = D // P                 # 8 contraction chunks of 128
NQT = S // 512              # 4 query tiles of 512 per batch
NTT = S // P                # 16 token tiles of 128 per batch
MASK_NEG = -480.0           # pre-scale mask add; *0.125 => -60 in the exponent

# matmul input dtype: float32r = single-pass fp32 (fast, ~bf16-grade mantissa)
# vs float32 = exact two-pass (4x slower). Toggled for accuracy/perf tradeoff.
FAST_MM = os.environ.get("KBENCH_EXACT_MM", "0") != "1"
MM_DT = mybir.dt.float32r if FAST_MM else mybir.dt.float32
F32 = mybir.dt.float32




def build_nc():
    """Build the per-core Bass program (SPMD: all cores run this, with
    per-core weight slices in their input maps)."""
    nc = bacc.Bacc("TRN2", target_bir_lowering=False, debug=False)

    xT = nc.dram_tensor("xT", [B, D, S], MM_DT, kind="ExternalInput")
    wq = nc.dram_tensor("wq", [D, P], MM_DT, kind="ExternalInput")
    wk = nc.dram_tensor("wk", [D, P], MM_DT, kind="ExternalInput")
    wv = nc.dram_tensor("wv", [D, P], MM_DT, kind="ExternalInput")
    wo = nc.dram_tensor("wo", [P, D], MM_DT, kind="ExternalInput")
    cosT = nc.dram_tensor("cosT", [B, P, S], F32, kind="ExternalInput")
    sinT = nc.dram_tensor("sinT", [B, P, S], F32, kind="ExternalInput")
    y = nc.dram_tensor("y", [B, S, D], F32, kind="ExternalOutput")

    with tile.TileContext(nc) as tc:
        _emit(nc, tc, xT, wq, wk, wv, wo, cosT, sinT, y)
    nc.compile()
    return nc


def _emit(nc, tc, xT, wq, wk, wv, wo, cosT, sinT, y):
    from contextlib import ExitStack

    ctx = ExitStack()
    with ctx:
        # ------------------------------------------------ pools
        singles = ctx.enter_context(tc.tile_pool(name="singles", bufs=1))
        xp = ctx.enter_context(tc.tile_pool(name="xp", bufs=CH))
        tabs = ctx.enter_context(tc.tile_pool(name="tabs", bufs=1))
        qkp = ctx.enter_context(tc.tile_pool(name="qkp", bufs=2))
        vtp = ctx.enter_context(tc.tile_pool(name="vtp", bufs=1))
        vp = ctx.enter_context(tc.tile_pool(name="vp", bufs=2))
        ropet = ctx.enter_context(tc.tile_pool(name="ropet", bufs=2))
        expp = ctx.enter_context(tc.tile_pool(name="expp", bufs=3))
        unp = ctx.enter_context(tc.tile_pool(name="unp", bufs=4))
        rrp = ctx.enter_context(tc.tile_pool(name="rrp", bufs=2))
        ysp = ctx.enter_context(tc.tile_pool(name="ysp", bufs=4))

        psA = ctx.enter_context(tc.tile_pool(name="psA", bufs=2, space="PSUM"))
        psB = ctx.enter_context(tc.tile_pool(name="psB", bufs=2, space="PSUM"))
        psC = ctx.enter_context(tc.tile_pool(name="psC", bufs=2, space="PSUM"))

        # ------------------------------------------------ constants
        # (memset/affine_select can't write f32r directly; build in f32 and
        # round via a DVE copy)
        ident_f = ropet.tile([P, P], F32, tag="t1", name="ident_f")
        make_identity(nc, ident_f)
        ident = singles.tile([P, P], MM_DT)
        nc.vector.tensor_copy(ident[:], ident_f[:])

        # mask_big[r, u] = 0 if u >= r + 384 else MASK_NEG   (u in [0, 896))
        mask_f = expp.tile([P, 896], F32, tag="e", name="mask_f")
        nc.gpsimd.memset(mask_f[:], 0.0)
        nc.gpsimd.affine_select(
            out=mask_f[:],
            in_=mask_f[:],
            compare_op=mybir.AluOpType.is_ge,
            fill=MASK_NEG,
            base=-384,
            pattern=[[1, 896]],
            channel_multiplier=-1,
        )
        mask_sb = singles.tile([P, 896], MM_DT)
        nc.vector.tensor_copy(mask_sb[:], mask_f[:])

        ones_sb = singles.tile([P, 1], F32)
        nc.vector.memset(ones_sb[:], 1.0)

        # weights: [D, 128] -> SBUF [128, CH, 128] (chunk c = rows c*128..)
        w_sbs = {}
        for nm, t in (("wq", wq), ("wk", wk), ("wv", wv)):
            w_sb = singles.tile([P, CH, P], MM_DT, name=f"{nm}_sb")
            nc.sync.dma_start(w_sb[:], t.ap().rearrange("(c p) m -> p c m", p=P))
            w_sbs[nm] = w_sb
        wo_sb = singles.tile([P, D], MM_DT)
        nc.sync.dma_start(wo_sb[:], wo[:])

        for b in range(B):
            # -------------------------------------------- load x^T, tables
            xc = []
            for c in range(CH):
                t = xp.tile([P, S], MM_DT, tag="xc", name=f"xc_{b}_{c}")
                nc.sync.dma_start(t[:], xT[b, c * P:(c + 1) * P, :])
                xc.append(t)
            cos_sb = tabs.tile([P, S], F32, tag="cos", name=f"cos_{b}")
            nc.sync.dma_start(cos_sb[:], cosT[b])
            spre_sb = tabs.tile([P, S], F32, tag="spre", name=f"spre_{b}")
            nc.sync.dma_start(spre_sb[:], sinT[b])

            # -------------------------------------------- projections
            q_sb = qkp.tile([P, S], MM_DT, tag="q", name=f"q_{b}")
            k_sb = qkp.tile([P, S], MM_DT, tag="k", name=f"k_{b}")
            vt_sb = vtp.tile([P, S], MM_DT, tag="vt", name=f"vt_{b}")

            for nm, dst in (("wq", q_sb), ("wk", k_sb), ("wv", vt_sb)):
                w_sb = w_sbs[nm]
                for jt in range(NQT):
                    js = jt * 512
                    pp = psC.tile([P, 512], F32, tag="u", name=f"pp_{nm}_{b}_{jt}")
                    for c in range(CH):
                        nc.tensor.matmul(
                            pp[:],
                            w_sb[:, c, :],
                            xc[c][:, js:js + 512],
                            start=(c == 0),
                            stop=(c == CH - 1),
                        )
                    if nm == "wv":
                        nc.vector.tensor_copy(dst[:, js:js + 512], pp[:])
                    else:
                        # RoPE: dst = pp*cos + blockswap32(pp*spre)
                        t1 = ropet.tile([P, 512], F32, tag="t1", name=f"t1_{nm}_{b}_{jt}")
                        nc.vector.tensor_mul(t1[:], pp[:], cos_sb[:, js:js + 512])
                        w2 = ropet.tile([P, 512], F32, tag="w2", name=f"w2_{nm}_{b}_{jt}")
                        nc.vector.tensor_mul(w2[:], pp[:], spre_sb[:, js:js + 512])
                        sh = ropet.tile([P, 512], F32, tag="sh", name=f"sh_{nm}_{b}_{jt}")
                        for blk in range(4):
                            src_blk = blk ^ 1  # swap 32-blocks within each 64
                            nc.sync.dma_start(
                                sh[blk * 32:(blk + 1) * 32, :],
                                w2[src_blk * 32:(src_blk + 1) * 32, :],
                            )
                        nc.vector.tensor_add(dst[:, js:js + 512], t1[:], sh[:])

            # -------------------------------------------- V transpose (+ones)
            # v_sb[:, tt, h*65 + 0:64] = V tokens, col h*65+64 = 1.0
            v_sb = vp.tile([P, NTT, 130], MM_DT, tag="v", name=f"v_{b}")
            nc.vector.tensor_copy(
                v_sb[:, :, 64::65], ones_sb[:, 0:1].to_broadcast([P, NTT, 2])
            )
            for tt in range(NTT):
                pt = psC.tile([P, 512], MM_DT, tag="u", name=f"pvt_{b}_{tt}")
                nc.tensor.transpose(
                    pt[:, 0:P], vt_sb[:, tt * P:(tt + 1) * P], ident[:]
                )
                nc.vector.tensor_copy(
                    v_sb[:, tt, :].rearrange("p (h c) -> p h c", h=2)[:, :, 0:64],
                    pt[:, 0:P].rearrange("p (h c) -> p h c", h=2),
                )

            # -------------------------------------------- attention
            un_tiles = []
            for qt in range(NQT):
                qs = qt * 512
                nkt = qt * 4 + 4
                ps_o = [
                    psB.tile([65, 512], F32, tag="o", name=f"po_{b}_{qt}_{h}")
                    for h in range(HLOC)
                ]
                for g in range(nkt // 2):
                    for h in range(HLOC):
                        sg = psA.tile([P, 1024], F32, tag="s", name=f"sg_{b}_{qt}_{g}_{h}")
                        for u in range(2):
                            kt = 2 * g + u
                            ks = kt * P
                            dlt = ks - qs
                            nc.tensor.matmul(
                                sg[:, u * 512:(u + 1) * 512],
                                k_sb[h * 64:h * 64 + 64, ks:ks + P],
                                q_sb[h * 64:h * 64 + 64, qs:qs + 512],
                                start=True,
                                stop=(dlt < 0),
                            )
                            if dlt >= 0:
                                nc.tensor.matmul(
                                    sg[:, u * 512:(u + 1) * 512],
                                    ident[:],
                                    mask_sb[:, 384 - dlt:896 - dlt],
                                    start=False,
                                    stop=True,
                                )
                        e = expp.tile([P, 1024], MM_DT, tag="e", name=f"e_{b}_{qt}_{g}_{h}")
                        nc.scalar.activation(
                            e[:], sg[:], mybir.ActivationFunctionType.Exp, scale=0.125
                        )
                        for u in range(2):
                            kt = 2 * g + u
                            nc.tensor.matmul(
                                ps_o[h][:],
                                v_sb[:, kt, h * 65:h * 65 + 65],
                                e[:, u * 512:(u + 1) * 512],
                                start=(kt == 0),
                                stop=(kt == nkt - 1),
                            )

                # normalize + stack heads: un [128 = 2x64 headdim, 512 tok]
                un = unp.tile([P, 512], MM_DT, tag="un", name=f"un_{b}_{qt}")
                for h in range(HLOC):
                    rr = rrp.tile([1, 512], F32, tag="rr", name=f"rr_{b}_{qt}_{h}")
                    nc.vector.reciprocal(rr[0:1, :], ps_o[h][64:65, :])
                    rb = rrp.tile([64, 512], F32, tag="rb", name=f"rb_{b}_{qt}_{h}")
                    nc.gpsimd.partition_broadcast(rb[:], rr[0:1, :])
                    nc.vector.tensor_mul(
                        un[h * 64:(h + 1) * 64, :], ps_o[h][0:64, :], rb[:]
                    )
                un_tiles.append(un)

            # -------------------------------------------- output projection
            for tt in range(NTT):
                un = un_tiles[tt // 4]
                tsl = slice((tt % 4) * P, (tt % 4) * P + P)
                for n in range(2):
                    yp = psC.tile([P, 512], F32, tag="u", name=f"yp_{b}_{tt}_{n}")
                    nc.tensor.matmul(
                        yp[:],
                        un[:, tsl],
                        wo_sb[:, n * 512:(n + 1) * 512],
                        start=True,
                        stop=True,
                    )
                    ys = ysp.tile([P, 512], F32, tag="ys", name=f"ys_{b}_{tt}_{n}")
                    if (tt + n) % 2 == 0:
                        nc.vector.tensor_copy(ys[:], yp[:])
                    else:
                        nc.scalar.copy(ys[:], yp[:])
                    nc.sync.dma_start(
                        y[b, tt * P:(tt + 1) * P, n * 512:(n + 1) * 512], ys[:]
                    )


# ------------------------------------------------------------------ host side

_PERM_HS = np.concatenate([np.arange(0, DK, 2), np.arange(1, DK, 2)])


def host_inputs(x, token_positions, Wq, Wk, Wv, Wo):
    """Build the shared + per-core device input maps."""
    x = np.asarray(x, dtype=np.float32)
    tp = np.asarray(token_positions)
    Wq = np.asarray(Wq, dtype=np.float32)
    Wk = np.asarray(Wk, dtype=np.float32)
    Wv = np.asarray(Wv, dtype=np.float32)
    Wo = np.asarray(Wo, dtype=np.float32)

    xT = np.ascontiguousarray(x.transpose(0, 2, 1))  # [B, D, S]

    # RoPE tables in the half-split + swapped-sin formulation
    inv_freq = (1.0 / (THETA ** (np.arange(0, DK, 2, dtype=np.float32) / DK))).astype(
        np.float32
    )  # [32]
    ang = tp.astype(np.float32)[:, None, :] * inv_freq[np.arange(P) % 32][None, :, None]
    cosT = np.cos(ang).astype(np.float32)  # [B, 128, S]
    sgn = np.where((np.arange(P) // 32) % 2 == 0, 1.0, -1.0).astype(np.float32)
    sinT = (np.sin(ang) * sgn[None, :, None]).astype(np.float32)

    in_maps = []
    for c in range(NCORES):
        heads = [c * HLOC + i for i in range(HLOC)]
        rows_hs = np.concatenate([h * DK + _PERM_HS for h in heads])   # q/k rows
        rows_pl = np.concatenate([h * DK + np.arange(DK) for h in heads])
        in_maps.append(
            {
                "xT": xT,
                "wq": np.ascontiguousarray(Wq[rows_hs].T),
                "wk": np.ascontiguousarray(Wk[rows_hs].T),
                "wv": np.ascontiguousarray(Wv[rows_pl].T),
                "wo": np.ascontiguousarray(Wo[:, rows_pl].T),
                "cosT": cosT,
                "sinT": sinT,
            }
        )
    return in_maps


_NC_CACHE = None


def kernel(x, token_positions, Wq, Wk, Wv, Wo, _want_results=False, **run_kwargs):
    """Full-input, full-output entry point. Shards across 8 NeuronCores."""
    global _NC_CACHE
    from concourse.bass_utils import run_bass_kernel_spmd

    in_maps = host_inputs(x, token_positions, Wq, Wk, Wv, Wo)
    if _NC_CACHE is None:
        _NC_CACHE = build_nc()
    res = run_bass_kernel_spmd(
        _NC_CACHE, in_maps, core_ids=list(range(NCORES)), **run_kwargs
    )
    out = np.zeros((B, S, D), dtype=np.float32)
    for r in res.results:
        out += r["y"]
    if _want_results:
        return out, res
    return out
